# revision 35
# baseline (speedup 1.0000x reference)
"""AttentionBlock3D kernel for 8 Trainium2 NeuronCores (Bass/Tile, SPMD).

Sharding: core c in 0..7 handles batch b = c//4 and query slice
qoff = (c%4)*512 of the N=2048 flattened positions. Each core computes the
GroupNorm affine + full K/V for its batch (replicated across the 4 cores
sharing a batch -> zero cross-core communication), attention for its 512
queries over all 2048 keys, projection and residual. Host gathers by pure
concatenation.

Key structure:
 - GroupNorm statistics are folded into per-channel affine coefficients on
   the host (same fp32 math as the reference; the host already precomputes
   exp(rel_pos_bias), the folded V bias and the weight transposes).
 - x is shipped in position-slab-major layout [4 slabs, C, 512].  The ramp
   pipelines per slab: affine -> K-tile-0 / Q-tile-0 contraction chunks ->
   PSUM->SBUF cast, so the first QK scores only need slab 0 + wq + wk
   (~2 MB of DMA) and the exp stream starts ~25us in (the fixed ~7us
   framework preamble + ~1.5us DMA-queue spin-up + HBM bandwidth bound
   the ramp), while the rest of x / bias / weights stream in behind it.
 - One ACT table set (exp) loaded once at t=0.
 - The K-projection bias is dropped: softmax_j((q+qb).(k_j+kb)) equals
   softmax_j((q+qb).k_j) exactly (q.kb terms are constant in j).  The q
   bias is applied by the DVE during the PSUM->SBUF cast of qT.
 - Bulk DMA is split across both hardware DGE queues (Sync + Activation);
   the small constants are packed into one [128, 24] tensor (one trigger).
 - Attention uses a transposed layout: scoresT[j, i] (keys on partitions,
   queries free) so the softmax denominator rides the AV matmul as a
   ones-column appended to V, and no transposes of the probability matrix
   are needed (scores are O(1): no max subtraction).  exp(scores) is
   multiplied by exp(bias) in one wide bf16 DVE op per key-group (the two
   heads of a pair share the bias via a broadcast access pattern).
 - A single pend queue delays AV matmuls 7 key-groups behind exp and is
   drained across head-pair boundaries; V chunks 6,7 and the second halves
   of K tiles 1..3 are emitted in the next head-pair's loop.  This keeps
   per-head-pair PE work close to the 18.4us of exp it must overlap (total
   PE stream time ~75us vs ~73us of exp on the scalar engine: the two
   engines are the co-bottlenecks and must stay in lock-step).
 - Tail: the output projection for channel chunks 0..2 is emitted before
   normalize(head-pair 3), so the PE never idles long enough for the HAM
   clock gate to re-throttle before the last matmuls.

Per-core inputs are rotated along the position axis by -qoff so that one
SPMD program (query slice = columns 0:512) serves all cores; GroupNorm and
softmax are permutation-invariant so results are unaffected.
"""
import sys

sys.path.insert(0, "/opt/trn_rl_repo")

from contextlib import ExitStack

import numpy as np

import concourse.bacc as bacc
import concourse.mybir as mybir
import concourse.tile as tile
from concourse.bass_utils import run_bass_kernel_spmd

B, C, D, H, W = 2, 512, 8, 16, 16
N = D * H * W  # 2048
HEADS, HD = 8, 64
GROUPS = 8
NUM_BUCKETS = 32
MAX_DIST = 128.0
EPS = 1e-5
NCORES = 8
NQ = N // 4  # 512 queries per core
F32 = mybir.dt.float32
F32R = mybir.dt.float32r
BF16 = mybir.dt.bfloat16

_CACHE = {}


def _build():
    nc = bacc.Bacc(
        "TRN2", target_bir_lowering=False, debug=False, num_devices=NCORES
    )
    AF = mybir.ActivationFunctionType
    OP = mybir.AluOpType

    xsl_d = nc.dram_tensor("xsl", [4, C, NQ], BF16, kind="ExternalInput").ap()
    xres_d = nc.dram_tensor("xres", [C, NQ], F32, kind="ExternalInput").ap()
    qkvwT_d = nc.dram_tensor("qkvwT", [C, 3 * C], BF16, kind="ExternalInput").ap()
    projwT_d = nc.dram_tensor("projwT", [C, C], BF16, kind="ExternalInput").ap()
    bias_d = nc.dram_tensor("expbT", [N, NQ], BF16, kind="ExternalInput").ap()
    # packed per-channel constants: [gna 0:4 | gnbv 4:8 | qkvb 8:20 | projb 20:24]
    cst_d = nc.dram_tensor("cst", [128, 24], F32, kind="ExternalInput").ap()
    out_d = nc.dram_tensor("out", [C, NQ], F32, kind="ExternalOutput").ap()

    with tile.TileContext(nc) as tc, ExitStack() as ctx:
        mb = ctx.enter_context(tc.tile_pool(name="mb", bufs=18))
        vg = ctx.enter_context(tc.tile_pool(name="vg", bufs=1))
        ex = ctx.enter_context(tc.tile_pool(name="ex", bufs=1))
        sm = ctx.enter_context(tc.tile_pool(name="sm", bufs=1))
        one = ctx.enter_context(tc.tile_pool(name="one", bufs=1))
        ps2 = ctx.enter_context(tc.tile_pool(name="ps2", bufs=1, space="PSUM"))
        ps1 = ctx.enter_context(tc.tile_pool(name="ps1", bufs=1, space="PSUM"))

        xh = [mb.tile([128, 4, NQ], BF16, tag="mb", name=f"xh{t}") for t in range(4)]
        wqkv = [
            mb.tile([128, 4, C], BF16, tag="mb", name=f"w{'qkv'[s]}")
            for s in range(3)
        ]
        wq, wk, wv = wqkv

        def load_w(ws, s, eng):
            eng.dma_start(
                out=ws,
                in_=qkvwT_d[:, C * s : C * (s + 1)].rearrange(
                    "(a p) o -> p a o", p=128
                ),
            )

        # --- sync queue: x slab 0, consts, then the rest of x + bias ---
        # --- scalar queue: wq, wk, wv, projwT -------------------------
        cst = one.tile([128, 24], F32)
        # sync queue: x slab 0 (4 triggers), consts, x slabs 1-3 per tile
        # (4 triggers), bias0.  scalar queue: wq, wk, wv, projwT.  gpsimd
        # SW-DGE queue: bias1-3, xres (late, uncontended).  Trigger issue
        # costs ~0.65us each on its engine, so the count is kept minimal.
        def load_x(t, eng):
            eng.dma_start(
                out=xh[t][:, 0, :], in_=xsl_d[0, 128 * t : 128 * (t + 1), :]
            )

        def load_x_rest(t, eng):
            eng.dma_start(
                out=xh[t][:, 1:4, :],
                in_=xsl_d[1:4, 128 * t : 128 * (t + 1), :].rearrange(
                    "s p i -> p s i"
                ),
            )

        bias_t = [
            mb.tile([128, 4, NQ], BF16, tag="mb", name=f"bias{q}") for q in range(4)
        ]

        def load_bias(q, eng):
            eng.dma_start(
                out=bias_t[q],
                in_=bias_d[512 * q : 512 * (q + 1), :].rearrange(
                    "(a p) i -> p a i", p=128
                ),
            )

        load_x(0, nc.sync)
        load_x(1, nc.sync)
        load_x(2, nc.scalar)
        load_x(3, nc.scalar)
        nc.sync.dma_start(out=cst, in_=cst_d)
        load_w(wq, 0, nc.scalar)
        load_w(wk, 1, nc.scalar)
        load_x_rest(0, nc.sync)
        load_x_rest(1, nc.sync)
        load_x_rest(2, nc.scalar)
        load_x_rest(3, nc.scalar)
        load_bias(0, nc.sync)
        load_w(wv, 2, nc.scalar)
        projwT = mb.tile([128, 4, C], BF16, tag="mb", name="projwT")
        xres = mb.tile([128, 4, NQ], F32, tag="big", bufs=2, name="xres")
        # projwT / bias1-3 / xres go on the gpsimd SW-DGE queue, gated
        # behind a dummy read of kt0 so their transfers don't steal HBM
        # bandwidth from the x/wq/wk front (emitted after the ramp below)

        # warm the exp table set (~1.3us) off the critical path
        warm = one.tile([1, 1], F32)
        nc.vector.memset(warm, 1.0)
        warm_eps = one.tile([1, 1], F32)
        nc.vector.memset(warm_eps, 0.0)
        nc.scalar.activation(
            out=warm, in_=warm, func=AF.Exp, bias=warm_eps, scale=1.0
        )

        ones8 = one.tile([128, HEADS], BF16)
        nc.vector.memset(ones8, 1.0)
        vaug = []
        for q in range(4):
            vt = vg.tile([128, 4, HEADS, 65], BF16, name=f"vaug{q}")
            nc.vector.tensor_copy(
                out=vt[:, :, :, 64:65].squeeze(3),
                in_=ones8.unsqueeze(1).broadcast_to([128, 4, HEADS]),
            )
            vaug.append(vt)

        # ---- ramp: per-slab GN affine + K0/Q0 chunks + casts --------
        qt = mb.tile([128, 4, NQ], BF16, tag="mb", name="qt")
        kt = {0: mb.tile([128, N], BF16, tag="mb", name="kt0")}
        # one pk tile reused for all four slabs (slab sl -> half sl%2; the
        # WAR against the previous slab's cast resolves early), and pq0 in
        # the ps_av pool: keeps two ps_s slots fresh for the first scores.
        pk0 = ps2.tile([128, 2, 512], F32, tag="ps_s", bufs=3, name="pk0")
        pq0 = ps1.tile([128, 512], F32, tag="ps_av", bufs=2, name="pq0")
        h_r = []
        for sl in range(4):
            for t in range(4):
                if sl == 0:
                    h_r.append(mb.tile([128, 4, NQ], BF16, tag="mb", name=f"h{t}"))
                nc.vector.tensor_scalar(
                    out=h_r[t][:, sl, :],
                    in0=xh[t][:, sl, :],
                    scalar1=cst[:, t : t + 1],
                    scalar2=cst[:, 4 + t : 5 + t],
                    op0=OP.mult,
                    op1=OP.add,
                )
            pk = pk0
            for ct in range(4):
                nc.tensor.matmul(
                    pk[:, sl % 2, :],
                    lhsT=wk[:, ct, 0:128],
                    rhs=h_r[ct][:, sl, :],
                    start=(ct == 0),
                    stop=(ct == 3),
                    skip_group_check=True,
                )
            if sl == 0:
                for ct in range(4):
                    nc.tensor.matmul(
                        pq0,
                        lhsT=wq[:, ct, 0:128],
                        rhs=h_r[ct][:, 0, :],
                        start=(ct == 0),
                        stop=(ct == 3),
                        skip_group_check=True,
                    )
            # per-slab cast (k bias cancels in softmax; q bias added on DVE)
            nc.vector.tensor_copy(
                out=kt[0][:, 512 * sl : 512 * (sl + 1)], in_=pk[:, sl % 2, :]
            )
            if sl == 0:
                nc.vector.tensor_scalar_add(
                    out=qt[:, 0, :], in0=pq0, scalar1=cst[:, 8:9]
                )

        # late bulk loads on the gpsimd SW-DGE queue.  Each destination
        # tile first gets a dummy one-element write that depends on the
        # kt0 cast (~15us): the WAW dependency stops the scheduler from
        # hoisting these DMAs into the bandwidth-critical ramp window.
        for dst in (projwT, bias_t[1], bias_t[2], bias_t[3]):
            nc.gpsimd.tensor_copy(
                out=dst[0:1, 0, 0:1], in_=kt[0][0:1, 0:1]
            )
        nc.gpsimd.tensor_copy(out=xres[0:1, 0, 0:1], in_=kt[0][0:1, 0:1])
        nc.gpsimd.dma_start(
            out=projwT, in_=projwT_d.rearrange("(a p) o -> p a o", p=128)
        )
        load_bias(1, nc.gpsimd)
        load_bias(2, nc.gpsimd)
        load_bias(3, nc.gpsimd)
        nc.gpsimd.dma_start(
            out=xres, in_=xres_d.rearrange("(a p) i -> p a i", p=128)
        )

        def emit_q_tile(ot):
            pq = ps2.tile([128, 512], F32, tag="ps_s", bufs=3, name=f"pq{ot}")
            for ct in range(4):
                nc.tensor.matmul(
                    pq,
                    lhsT=wq[:, ct, 128 * ot : 128 * (ot + 1)],
                    rhs=h_r[ct][:, 0, :],
                    start=(ct == 0),
                    stop=(ct == 3),
                    skip_group_check=True,
                )
            nc.vector.tensor_scalar_add(
                out=qt[:, ot, :], in0=pq, scalar1=cst[:, 8 + ot : 9 + ot]
            )

        def emit_kt_half(ot, njp):
            if njp == 0:
                kt[ot] = mb.tile([128, N], BF16, tag="mb", name=f"kt{ot}")
            pk = ps2.tile(
                [128, 2, 512], F32, tag="ps_s", bufs=3, name=f"pk{ot}{njp}"
            )
            for nh in range(2):
                for ct in range(4):
                    nc.tensor.matmul(
                        pk[:, nh, :],
                        lhsT=wk[:, ct, 128 * ot : 128 * (ot + 1)],
                        rhs=h_r[ct][:, 2 * njp + nh, :],
                        start=(ct == 0),
                        stop=(ct == 3),
                        skip_group_check=True,
                    )
            nc.vector.tensor_copy(
                out=kt[ot][:, 1024 * njp : 1024 * (njp + 1)],
                in_=pk.rearrange("p a i -> p (a i)"),
            )

        def emit_v_chunk(ntp):
            pv = ps2.tile([128, 2, 512], F32, tag="ps_s", bufs=3, name=f"pv{ntp}")
            for nh in range(2):
                nt = 2 * ntp + nh
                for ct in range(4):
                    nc.tensor.matmul(
                        pv[:, nh, :],
                        lhsT=h_r[ct][:, nt // 4, 128 * (nt % 4) : 128 * (nt % 4 + 1)],
                        rhs=wv[:, ct, :],
                        start=(ct == 0),
                        stop=(ct == 3),
                        skip_group_check=True,
                    )
            q, jj = (2 * ntp) // 4, (2 * ntp) % 4
            nc.vector.tensor_copy(
                out=vaug[q][:, jj : jj + 2, :, 0:64],
                in_=pv.rearrange("p a (h d) -> p a h d", d=HD),
            )

        # ---- attention: flat (hp, g) stream with one global pend queue --
        # attnT as 4 per-ct tiles: the half-row DMA writers would otherwise
        # create whole-tile false deps that stall the tail projection
        attnT = [
            mb.tile([128, NQ], BF16, tag="at", bufs=4, name=f"attnT{ct}")
            for ct in range(4)
        ]
        av_of = {}

        def emit_av(entry):
            hp, gp, etp = entry
            ha, hb = 2 * hp, 2 * hp + 1
            for h, hi in ((ha, 0), (hb, 1)):
                for jj in range(2):
                    jb = 2 * gp + jj
                    nc.tensor.matmul(
                        av_of[h][0:65, :],
                        lhsT=vaug[jb // 4][:, jb % 4, h, :],
                        rhs=etp[:, hi, jj, :],
                        start=(gp == 0 and jj == 0),
                        stop=(gp == 7 and jj == 1),
                        skip_group_check=True,
                    )

        def normalize(hp):
            # rows 0:63 = unnormalized attn^T, row 64 = denom.  For the
            # last head-pair the odd head goes first: its cross-partition
            # DMA is on the tail critical path.
            heads = (2 * hp, 2 * hp + 1) if hp < 3 else (7, 6)
            for h in heads:
                dsb = sm.tile([1, 512], F32, tag="den", bufs=4, name=f"den{h}")
                nc.vector.tensor_copy(out=dsb, in_=av_of[h][64:65, :])
                denr = sm.tile([1, 512], F32, tag="denr", bufs=4, name=f"dr{h}")
                nc.vector.reciprocal_approx_fast(out=denr, in_=dsb)
                den_bc = sm.tile(
                    [64, 512], F32, tag="den_bc", bufs=2, name=f"dbc{h}"
                )
                nc.gpsimd.partition_broadcast(out_ap=den_bc, in_ap=denr)
                if h % 2 == 0:
                    nc.vector.tensor_tensor(
                        out=attnT[h // 2][0:64, :],
                        in0=av_of[h][0:64, :],
                        in1=den_bc,
                        op=OP.mult,
                    )
                else:
                    half = sm.tile(
                        [64, 512], BF16, tag="half", bufs=2, name=f"hf{h}"
                    )
                    nc.vector.tensor_tensor(
                        out=half, in0=av_of[h][0:64, :], in1=den_bc, op=OP.mult
                    )
                    eng = nc.scalar if hp == 3 else nc.sync
                    eng.dma_start(out=attnT[h // 2][64:128, :], in_=half)

        pend = []  # delayed AV emission: (hp, g, et) across hp boundaries
        it = 0
        for hp in range(4):
            ha, hb = 2 * hp, 2 * hp + 1
            av_of[ha] = ps1.tile(
                [128, 512], F32, tag="ps_av", bufs=2, name=f"av{ha}"
            )
            av_of[hb] = ps1.tile(
                [128, 512], F32, tag="ps_av", bufs=2, name=f"av{hb}"
            )
            for g in range(8):
                TA = ps2.tile(
                    [128, 2, 512], F32, tag="ps_s", bufs=3, name=f"sa{hp}_{g}"
                )
                TB = ps2.tile(
                    [128, 2, 512], F32, tag="ps_s", bufs=3, name=f"sb{hp}_{g}"
                )
                for jj in range(2):
                    jb = 2 * g + jj
                    js = slice(128 * jb, 128 * (jb + 1))
                    # the two K=64 matmuls run concurrently (row groups 0/64)
                    nc.tensor.matmul(
                        TA[:, jj, :],
                        lhsT=kt[hp][0:64, js],
                        rhs=qt[0:64, hp, :],
                        start=True,
                        stop=True,
                        skip_group_check=True,
                    )
                    nc.tensor.matmul(
                        TB[:, jj, :],
                        lhsT=kt[hp][64:128, js],
                        rhs=qt[64:128, hp, :],
                        start=True,
                        stop=True,
                        skip_group_check=True,
                    )
                etr = ex.tile(
                    [128, 2, 2, 512], BF16, tag="etr", bufs=5, name=f"er{hp}_{g}"
                )
                nc.scalar.activation(
                    out=etr[:, 0, :, :], in_=TA, func=AF.Exp, scale=0.125
                )
                nc.scalar.activation(
                    out=etr[:, 1, :, :], in_=TB, func=AF.Exp, scale=0.125
                )
                et = ex.tile(
                    [128, 2, 2, 512], BF16, tag="et", bufs=10, name=f"et{hp}_{g}"
                )
                jb0 = 2 * g
                nc.vector.tensor_tensor(
                    out=et,
                    in0=etr,
                    in1=bias_t[jb0 // 4][:, jb0 % 4 : jb0 % 4 + 2, :]
                    .unsqueeze(1)
                    .broadcast_to([128, 2, 2, 512]),
                    op=OP.mult,
                )
                pend.append((hp, g, et))
                # filler PE work AFTER this iteration's QK so the exp
                # stream is never starved at head-pair boundaries
                if 0 <= it - 2 <= 7:
                    emit_v_chunk(it - 2)  # chunks 0..7 at iters 2..9
                if g == 1 and hp > 0:
                    emit_kt_half(hp, 1)
                # later K/Q tiles (q first: the kt tile allocation's WAR
                # against wq must resolve forward)
                if g == 3 and hp < 3:
                    emit_q_tile(hp + 1)
                if g == 4 and hp < 3:
                    emit_kt_half(hp + 1, 0)
                while len(pend) > 8:
                    entry = pend.pop(0)
                    emit_av(entry)
                    if entry[1] == 7:
                        normalize(entry[0])
                it += 1
        for entry in pend:
            emit_av(entry)

        # ---- projection + residual ----------------------------------
        # ct 0..2 are final already; emit their projection matmuls before
        # normalize(3) so the PE stays busy (and warm) through the tail.
        outsb = mb.tile([128, 4, NQ], F32, tag="big", bufs=2, name="outsb")
        pp01 = ps2.tile([128, 2, 512], F32, tag="ps_s", bufs=3, name="pp01")
        pp23 = ps2.tile([128, 2, 512], F32, tag="ps_s", bufs=3, name="pp23")

        def proj_ct(ct):
            for ot in range(4):
                pp = pp01 if ot < 2 else pp23
                nc.tensor.matmul(
                    pp[:, ot % 2, :],
                    lhsT=projwT[:, ct, 128 * ot : 128 * (ot + 1)],
                    rhs=attnT[ct],
                    start=(ct == 0),
                    stop=(ct == 3),
                    skip_group_check=True,
                )

        for ct in range(3):
            proj_ct(ct)
        normalize(3)
        proj_ct(3)
        for ot in range(4):
            pp = pp01 if ot < 2 else pp23
            nc.vector.scalar_tensor_tensor(
                out=outsb[:, ot, :],
                in0=pp[:, ot % 2, :],
                scalar=cst[:, 20 + ot : 21 + ot],
                in1=xres[:, ot, :],
                op0=OP.add,
                op1=OP.add,
            )
            nc.sync.dma_start(
                out=out_d[128 * ot : 128 * (ot + 1), :], in_=outsb[:, ot, :]
            )

    nc.finalize()
    return nc


def _host_prep(x, gn_w, gn_b, qkv_w, qkv_b, proj_w, proj_b, rel_emb):
    """Build the 8 per-core input maps."""
    x = np.asarray(x, dtype=np.float32)
    gn_w = np.asarray(gn_w, dtype=np.float32)
    gn_b = np.asarray(gn_b, dtype=np.float32)
    qkv_w = np.asarray(qkv_w, dtype=np.float32)
    qkv_b = np.asarray(qkv_b, dtype=np.float32)
    proj_w = np.asarray(proj_w, dtype=np.float32)
    proj_b = np.asarray(proj_b, dtype=np.float32)
    rel_emb = np.asarray(rel_emb, dtype=np.float32)

    # relative position bias (matches reference._rel_pos_bias, float32 math)
    dd, hh, ww = np.meshgrid(
        np.arange(D), np.arange(H), np.arange(W), indexing="ij"
    )
    coords = np.stack(
        [dd.ravel(), hh.ravel(), ww.ravel()], axis=-1
    ).astype(np.float32)
    rel = coords[:, None, :] - coords[None, :, :]
    dist = np.sqrt(np.sum(rel * rel, axis=-1, dtype=np.float32)).astype(np.float32)
    buckets = np.clip(
        np.floor(dist / np.float32(MAX_DIST / NUM_BUCKETS)).astype(np.int32),
        0,
        NUM_BUCKETS - 1,
    )
    expb = np.exp(rel_emb[buckets]).astype(np.float32)  # [N, N], symmetric

    import ml_dtypes

    bf16 = ml_dtypes.bfloat16
    projb_eff = (proj_b + proj_w @ qkv_b[2 * C : 3 * C]).astype(np.float32)
    qkvwT = np.ascontiguousarray(qkv_w.T).astype(bf16)
    projwT = np.ascontiguousarray(proj_w.T).astype(bf16)

    xb = x.reshape(B, C, N)
    # GroupNorm statistics (fp32, identical math to the reference) folded
    # into per-channel affine coefficients per batch
    cst_b = []
    for b in range(B):
        xg = xb[b].reshape(GROUPS, (C // GROUPS) * N)
        mu = xg.mean(axis=1)
        var = xg.var(axis=1)
        rstd = 1.0 / np.sqrt(var + np.float32(EPS))
        a_c = (gn_w * rstd[np.arange(C) // (C // GROUPS)]).astype(np.float32)
        b_c = (gn_b - mu[np.arange(C) // (C // GROUPS)] * a_c).astype(np.float32)
        cst = np.zeros((128, 24), np.float32)
        cst[:, 0:4] = a_c.reshape(4, 128).T
        cst[:, 4:8] = b_c.reshape(4, 128).T
        cst[:, 8:20] = qkv_b.reshape(3, 4, 128).transpose(2, 0, 1).reshape(128, 12)
        cst[:, 20:24] = projb_eff.reshape(4, 128).T
        cst_b.append(cst)

    in_maps = []
    for c in range(NCORES):
        b, qoff = c // 4, (c % 4) * NQ
        xroll = np.roll(xb[b], -qoff, axis=1)
        # position-slab-major x: [4, C, 512]
        xsl = np.ascontiguousarray(
            xroll.reshape(C, 4, NQ).transpose(1, 0, 2)
        ).astype(bf16)
        xres_c = np.ascontiguousarray(xroll[:, 0:NQ])
        bias_c = np.ascontiguousarray(
            np.roll(expb, -qoff, axis=0)[:, qoff : qoff + NQ]
        ).astype(bf16)
        in_maps.append(
            {
                "xsl": xsl,
                "xres": xres_c,
                "qkvwT": qkvwT,
                "projwT": projwT,
                "expbT": bias_c,
                "cst": cst_b[b],
            }
        )
    return in_maps


def _run(inputs, trace=False, trace_cores=None):
    if "nc" not in _CACHE:
        _CACHE["nc"] = _build()
    nc = _CACHE["nc"]
    in_maps = _host_prep(**inputs)
    last_err = None
    for attempt in range(3):
        try:
            res = run_bass_kernel_spmd(
                nc,
                in_maps,
                core_ids=list(range(NCORES)),
                trace=trace,
                trace_cores=trace_cores,
            )
            break
        except Exception as e:  # transient NRT device errors on first exec
            last_err = e
            import time as _time

            _time.sleep(2.0)
            try:
                import jax

                jax.clear_backends()
            except Exception:
                pass
    else:
        raise last_err
    out = np.empty((B, C, N), np.float32)
    for c in range(NCORES):
        b, qoff = c // 4, (c % 4) * NQ
        out[b][:, qoff : qoff + NQ] = res.results[c]["out"]
    return out.reshape(B, C, D, H, W), res


def kernel(**inputs) -> np.ndarray:
    out, _ = _run(inputs, trace=False)
    return out
